# revision 25
# baseline (speedup 1.0000x reference)
"""Trainium2 Bass kernel for the AttentionBlock problem.

Reference semantics (shapes hardcoded):
    x [4, 256, 64, 64]; 1x1-conv weights q_w/k_w/v_w [256, 258] (+biases),
    fc_w [256, 256], fc_b [256].
    x0 = concat(x, pos) -> [B, 258, 4096]
    q/k/v = relu(W @ x0 + b)                    [B, 256, 4096]
    attn  = softmax_causal(q^T k)               [B, 4096, 4096]
    out   = x + relu(fc_w @ (attn @ v^T)^T + fc_b)

Distribution: 8 cores = 4 batches x 2 query-block roles. Each core
computes full k / v^T for its batch, q only for its 4 owned 512-wide
query blocks, and causal attention for those blocks. Causal work is
balanced by giving role 0 global blocks [0,3,4,7] and role 1 blocks
[1,2,5,6]; both roles run the identical SPMD program with per-slot
key-tile counts [8,16,24,32] (slightly padded); per-core mask data
zeroes padded/non-causal entries.

The whole kernel runs in bf16 (PSUM f32): measured on this hardware,
bf16 and f32r matmuls issue at the same rate, but f32r weight loads
are 2x slower and f32r<->bf16 switches cost ~250-450ns, so all-bf16
both removes every switch and halves weight-load time.  Host-sim
error of the all-bf16 path is ~0.9e-2 vs the 2e-2 tolerance.
Softmax runs without max-subtraction (scores ~20..67, far below
overflow); denominator via a replicated ones-matmul of quad sums.
Engine placement (measured: GpSimd is pathologically slow - avoid):
ScalarE exp + k/q/fc relus, VectorE v relus/masks/quads/normalize/
residual, PE all matmuls including the rank-3 pos+bias accumulation
terms.  Input DMAs are split across the two HWDGE queues (sync +
scalar engines) and x0 is resident, transferred in per-pair column
chunks so the first projections start as early as possible.

Measured on this problem's hardware: 8 cores, ~142 us vs the 183 us
f32r/bf16 baseline; dominated by Tensor-engine busy time (~116 us at
~97% issue efficiency).  fp8e4m3 DoubleRow attn@v was implemented and
passed correctness (rel err 1.29e-2) but lost end-to-end: the
per-query rescale into fp8 costs ~1.6 us per [128,512] tile on DVE
(fp8 writes fall off the 2x path), exceeding the PE savings.
"""

import numpy as np

B = 4
C = 256
S = 64
N = S * S            # 4096
K = 256              # q/k/v channels
NBLK = 512           # query block width
NSLOT = 4            # owned query blocks per core
M_S = (8, 16, 24, 32)  # key-tile count per slot (128-wide key tiles)
BLOCKS = ((0, 3, 4, 7), (1, 2, 5, 6))  # role -> global block ids

_PROGRAM = None


def _build_program():
    import concourse.bacc as bacc
    import concourse.mybir as mybir
    import concourse.tile as tile

    F32 = mybir.dt.float32
    BF16 = mybir.dt.bfloat16
    Act = mybir.ActivationFunctionType

    nc = bacc.Bacc("TRN2", target_bir_lowering=False, debug=False)

    x0b_d = nc.dram_tensor("x0b", [C, N], BF16, kind="ExternalInput")
    x0p_d = nc.dram_tensor("x0p", [128, N], BF16, kind="ExternalInput")
    x0c_d = nc.dram_tensor("x0c", [3, N], BF16, kind="ExternalInput")
    xq_d = nc.dram_tensor("xq", [C, NSLOT * NBLK], BF16, kind="ExternalInput")
    xqp_d = nc.dram_tensor("xqp", [128, NSLOT * NBLK], BF16,
                           kind="ExternalInput")
    wq_d = nc.dram_tensor("wq", [C, K], BF16, kind="ExternalInput")
    wk_d = nc.dram_tensor("wk", [C, K], BF16, kind="ExternalInput")
    wv_d = nc.dram_tensor("wv", [C, K], BF16, kind="ExternalInput")
    wqp_d = nc.dram_tensor("wqp", [128, K], BF16, kind="ExternalInput")
    wkp_d = nc.dram_tensor("wkp", [128, K], BF16, kind="ExternalInput")
    wvc_d = nc.dram_tensor("wvc", [3, K], BF16, kind="ExternalInput")
    fcw_d = nc.dram_tensor("fcw", [C, C], BF16, kind="ExternalInput")
    fcb_d = nc.dram_tensor("fcb", [C, 1], F32, kind="ExternalInput")
    msk_d = nc.dram_tensor("masks", [NSLOT, 8, 128, NBLK], BF16,
                           kind="ExternalInput")
    od_d = nc.dram_tensor("onesd", [128, 128], BF16, kind="ExternalInput")
    xres_d = nc.dram_tensor("xres", [C, NSLOT * NBLK], F32,
                            kind="ExternalInput")
    out_d = nc.dram_tensor("out", [C, NSLOT * NBLK], F32, kind="ExternalOutput")

    with tile.TileContext(nc) as tc:
        with (
            tc.tile_pool(name="wts", bufs=1) as wts,
            tc.tile_pool(name="kqv_p", bufs=1) as kqv_p,
            tc.tile_pool(name="msk_p", bufs=6) as msk_p,
            tc.tile_pool(name="ex_p", bufs=10) as ex_p,
            tc.tile_pool(name="ds_p", bufs=4) as ds_p,
            tc.tile_pool(name="f_p", bufs=2) as f_p,
            tc.tile_pool(name="o_p", bufs=2) as o_p,
            tc.tile_pool(name="tr_p", bufs=2) as tr_p,
            tc.tile_pool(name="ps_sc", bufs=5, space="PSUM") as ps_sc,
            tc.tile_pool(name="ps_dfc", bufs=1, space="PSUM") as ps_dfc,
            tc.tile_pool(name="ps_o", bufs=1, space="PSUM") as ps_o,
        ):
            def wtile(dram, r0, rn, dt, tag, eng=None):
                t = wts.tile([rn, dram.shape[1]], dt, tag=tag, name=tag)
                (eng or nc.sync).dma_start(t[:], dram[r0:r0 + rn, :])
                return t

            # resident x0 (+pos pad), DMAed in per-pair column chunks so
            # the first projections are not gated on the full transfer
            x0_t = [kqv_p.tile([128, N], BF16, tag=f"x0{ci}", name=f"x0{ci}")
                    for ci in range(2)]
            x0p_t = kqv_p.tile([128, N], BF16, tag="x0p", name="x0p")
            x0c_t = kqv_p.tile([3, N], BF16, tag="x0c", name="x0c")

            # phase-A-first weights (k, v) so PE can start early
            wk_t = [wtile(wk_d, 0, 128, BF16, "wk0"),
                    wtile(wk_d, 128, 128, BF16, "wk1"),
                    wtile(wkp_d, 0, 128, BF16, "wk2")]

            def chunk_dma(c0, cn):
                sl = slice(c0, c0 + cn)
                nc.sync.dma_start(x0_t[0][:, sl], x0b_d[0:128, sl])
                nc.scalar.dma_start(x0_t[1][:, sl], x0b_d[128:256, sl])
                nc.scalar.dma_start(x0p_t[:, sl], x0p_d[:, sl])
                nc.sync.dma_start(x0c_t[:, sl], x0c_d[:, sl])

            # first 512 columns only, so nb-0's k matmuls start ASAP
            chunk_dma(0, 512)
            chunk_dma(512, 512)
            wv_t = [wtile(wv_d, 0, 128, BF16, "wv0", eng=nc.scalar),
                    wtile(wv_d, 128, 128, BF16, "wv1", eng=nc.scalar),
                    wtile(wvc_d, 0, 3, BF16, "wv2", eng=nc.scalar)]

            k_sb = [[None] * 8 for _ in range(2)]
            vT_sb = [None] * 32

            def emit_pair(nbp):
                for nb in (2 * nbp, 2 * nbp + 1):
                    sl = slice(NBLK * nb, NBLK * (nb + 1))
                    for kt in range(2):
                        kts = slice(128 * kt, 128 * (kt + 1))
                        pk = ps_sc.tile([128, NBLK], F32, tag="sc",
                                        name=f"pk{kt}_{nb}")
                        nc.tensor.matmul(pk[:], wk_t[0][:, kts],
                                         x0_t[0][:, sl], start=True,
                                         stop=False)
                        nc.tensor.matmul(pk[:], wk_t[1][:, kts],
                                         x0_t[1][:, sl], start=False,
                                         stop=False)
                        nc.tensor.matmul(pk[:], wk_t[2][:, kts],
                                         x0p_t[:, sl], start=False,
                                         stop=True)
                        kt_sb = kqv_p.tile([128, NBLK], BF16,
                                           tag=f"k{kt}_{nb}",
                                           name=f"k{kt}_{nb}")
                        nc.scalar.activation(kt_sb[:], pk[:], Act.Relu)
                        k_sb[kt][nb] = kt_sb
                for nb in (2 * nbp, 2 * nbp + 1):
                    for sub in range(4):
                        i = 4 * nb + sub
                        ss = slice(128 * i, 128 * (i + 1))
                        pv = ps_sc.tile([128, K], F32, tag="sc",
                                        name=f"pv{i}")
                        nc.tensor.matmul(pv[:], x0_t[0][:, ss], wv_t[0][:],
                                         start=True, stop=False)
                        nc.tensor.matmul(pv[:], x0_t[1][:, ss], wv_t[1][:],
                                         start=False, stop=False)
                        nc.tensor.matmul(pv[:], x0c_t[:, ss], wv_t[2][:],
                                         start=False, stop=True)
                        vt_sb = kqv_p.tile([128, K], BF16, tag=f"v{i}",
                                           name=f"v{i}")
                        nc.vector.tensor_scalar_max(vt_sb[:], pv[:], 0.0)
                        vT_sb[i] = vt_sb

            q_sb = [[None] * NSLOT for _ in range(2)]

            def emit_q(s):
                sl = slice(NBLK * s, NBLK * (s + 1))
                for kt in range(2):
                    kts = slice(128 * kt, 128 * (kt + 1))
                    pq = ps_sc.tile([128, NBLK], F32, tag="sc",
                                    name=f"pq{kt}_{s}")
                    nc.tensor.matmul(pq[:], wq_t[0][:, kts], xq_t[0][:, sl],
                                     start=True, stop=False)
                    nc.tensor.matmul(pq[:], wq_t[1][:, kts], xq_t[1][:, sl],
                                     start=False, stop=False)
                    nc.tensor.matmul(pq[:], wq_t[2][:, kts], xqp_t[:, sl],
                                     start=False, stop=True)
                    qt = kqv_p.tile([128, NBLK], BF16, tag=f"q{kt}_{s}",
                                    name=f"q{kt}_{s}")
                    nc.scalar.activation(qt[:], pq[:], Act.Relu)
                    q_sb[kt][s] = qt

            slot_state = {}  # s -> (po, pd)

            def emit_slot(s, pending=None):
                """scores, exp, masks, den quads, attn@v for slot s;
                finalize(pending) is emitted after the first quad so its
                PSUM banks free early and the fc work hides in this
                slot's score stream."""
                M = M_S[s]
                po = [ps_o.tile([128, NBLK], F32, tag=f"o{vt}",
                                name=f"po{vt}_{s}") for vt in range(2)]
                pd = ps_dfc.tile([128, NBLK], F32, tag="dfc", name=f"pd{s}")
                ex_tiles = [None] * M
                for i in range(M):
                    psc = ps_sc.tile([128, NBLK], F32, tag="sc",
                                     name=f"psc{s}_{i}")
                    for kt in range(2):
                        nc.tensor.matmul(
                            psc[:],
                            k_sb[kt][i // 4][:, 128 * (i % 4):128 * (i % 4 + 1)],
                            q_sb[kt][s][:], start=(kt == 0), stop=(kt == 1))
                    ex = ex_p.tile([128, NBLK], BF16, tag="ex",
                                   name=f"ex{s}_{i}")
                    nc.scalar.activation(ex[:], psc[:], Act.Exp)
                    if i >= M - 8:
                        mk = msk_p.tile([128, NBLK], BF16, tag="mk",
                                        name=f"mk{s}_{i}")
                        nc.scalar.dma_start(mk[:], msk_d[s, i - (M - 8)])
                        nc.vector.tensor_mul(ex[:], ex[:], mk[:])
                    ex_tiles[i] = ex
                    if i % 4 == 3:
                        j = i - 3
                        if j == 0 and pending is not None:
                            finalize(pending)
                        # attn@v for the finished quad
                        for jj in range(j, j + 4):
                            e = ex_tiles[jj]
                            for vt in range(2):
                                nc.tensor.matmul(
                                    po[vt][:],
                                    vT_sb[jj][:, 128 * vt:128 * (vt + 1)],
                                    e[:], start=(jj == 0), stop=(jj == M - 1))
                        # quad-summed denominator
                        da = ds_p.tile([128, NBLK], BF16, tag="ds",
                                       name=f"da{s}_{j}")
                        nc.vector.tensor_add(da[:], ex_tiles[j][:],
                                             ex_tiles[j + 1][:])
                        db = ds_p.tile([128, NBLK], BF16, tag="ds",
                                       name=f"db{s}_{j}")
                        nc.vector.tensor_add(db[:], ex_tiles[j + 2][:],
                                             ex_tiles[j + 3][:])
                        dsum = ds_p.tile([128, NBLK], BF16, tag="ds",
                                         name=f"ds{s}_{j}")
                        nc.vector.tensor_add(dsum[:], da[:], db[:])
                        nc.tensor.matmul(pd[:], od_t[:], dsum[:],
                                         start=(j == 0), stop=(j == M - 4))
                        for jj in range(j, j + 4):
                            ex_tiles[jj] = None
                slot_state[s] = (po, pd)

            def finalize(s, halves=1):
                """normalize, fc, relu, residual, dma out for slot s.
                halves=2 pipelines the chain in 256-col pieces (used on
                the last slot, where this chain is the kernel tail)."""
                po, pd = slot_state[s]
                hw = NBLK // halves
                for h in range(halves):
                    hs = slice(h * hw, (h + 1) * hw)
                    rb = f_p.tile([128, hw], F32, tag="rb",
                                  name=f"rb{s}_{h}")
                    nc.vector.reciprocal_approx_fast(rb[:], pd[:, hs])
                    o_sb = []
                    for vt in range(2):
                        ot = o_p.tile([128, hw], BF16, tag=f"ob{vt}",
                                      name=f"ob{vt}_{s}_{h}")
                        nc.vector.tensor_mul(ot[:], po[vt][:, hs], rb[:])
                        o_sb.append(ot)
                    for ot in range(2):
                        pfc = ps_dfc.tile([128, hw], F32, tag="dfc",
                                          name=f"pfc{ot}_{s}_{h}")
                        for vt in range(2):
                            nc.tensor.matmul(
                                pfc[:],
                                fcw_t[vt][:, 128 * ot:128 * (ot + 1)],
                                o_sb[vt][:], start=(vt == 0),
                                stop=(vt == 1))
                        t_sb = tr_p.tile([128, hw], F32, tag=f"t{ot}",
                                         name=f"t{ot}_{s}_{h}")
                        nc.scalar.activation(t_sb[:], pfc[:], Act.Relu,
                                             bias=fcb_t[ot][:])
                        r_sb = tr_p.tile([128, hw], F32, tag=f"r{ot}",
                                         name=f"r{ot}_{s}_{h}")
                        nc.vector.tensor_add(
                            r_sb[:], t_sb[:],
                            xres_t[ot][:, NBLK * s + h * hw:
                                        NBLK * s + (h + 1) * hw])
                        nc.sync.dma_start(
                            out_d[128 * ot:128 * (ot + 1),
                                  NBLK * s + h * hw:
                                  NBLK * s + (h + 1) * hw], r_sb[:])

            # ---- emission schedule: spread k/v pairs between slots so
            # DMA and projections overlap attention; all bf16, no dtype
            # switches anywhere.
            emit_pair(0)

            # q inputs next on the queues (q runs right after pair 0) ...
            wq_t = [wtile(wq_d, 0, 128, BF16, "wq0"),
                    wtile(wq_d, 128, 128, BF16, "wq1"),
                    wtile(wqp_d, 0, 128, BF16, "wq2")]
            xq_t = [wtile(xq_d, 0, 128, BF16, "xq0"),
                    wtile(xq_d, 128, 128, BF16, "xq1")]
            xqp_t = wtile(xqp_d, 0, 128, BF16, "xqp")
            fcb_t = [wtile(fcb_d, 0, 128, F32, "fcb0"),
                     wtile(fcb_d, 128, 128, F32, "fcb1")]
            od_t = wtile(od_d, 0, 128, BF16, "onesd")

            emit_q(0)
            emit_q(1)
            emit_slot(0)
            chunk_dma(1024, 1024)
            fcw_t = [wtile(fcw_d, 0, 128, BF16, "fcw0", eng=nc.scalar),
                     wtile(fcw_d, 128, 128, BF16, "fcw1", eng=nc.scalar)]
            xres_t = [wtile(xres_d, 128 * ot, 128, F32, f"xres{ot}",
                            eng=nc.scalar) for ot in range(2)]
            emit_pair(1)
            emit_q(2)
            emit_slot(1, pending=0)
            chunk_dma(2048, 1024)
            emit_pair(2)
            emit_q(3)
            emit_slot(2, pending=1)
            chunk_dma(3072, 1024)
            emit_pair(3)
            emit_slot(3, pending=2)
            finalize(3, halves=2)

    nc.compile()
    return nc


def _host_prep(x, q_w, q_b, k_w, k_b, v_w, v_b, fc_w, fc_b):
    """Build the per-core input maps."""
    import ml_dtypes
    f32 = np.float32
    bf16 = ml_dtypes.bfloat16
    n = np.arange(N)
    px = ((n // S) / S).astype(f32)
    py = ((n % S) / S).astype(f32)
    pos3 = np.stack([px, py, np.ones(N, f32)])   # [3, N] incl bias channel

    pos_pad = np.zeros((128, N), f32)
    pos_pad[:3] = pos3

    def pad_w(w, b):
        # rows 0..1 = pos weight rows, row 2 = bias, rest zero
        p = np.zeros((128, K), f32)
        p[:2] = w.astype(f32).T[C:]
        p[2] = b.astype(f32)
        return p

    # per-role masks [NSLOT, 8, 128, 512]
    mm = np.arange(128)[:, None]
    nn = np.arange(NBLK)[None, :]
    masks = {}
    for r in range(2):
        mr = np.zeros((NSLOT, 8, 128, NBLK), f32)
        for s in range(NSLOT):
            j = BLOCKS[r][s]
            for t in range(8):
                i = M_S[s] - 8 + t
                mr[s, t] = (128 * i + mm <= 512 * j + nn)
        masks[r] = mr.astype(bf16)

    shared = {
        "wq": np.ascontiguousarray(q_w.astype(f32).T[:C]).astype(bf16),
        "wk": np.ascontiguousarray(k_w.astype(f32).T[:C]).astype(bf16),
        "wv": np.ascontiguousarray(v_w.astype(f32).T[:C]).astype(bf16),
        "wqp": pad_w(q_w, q_b).astype(bf16),
        "wkp": pad_w(k_w, k_b).astype(bf16),
        "wvc": np.ascontiguousarray(
            np.concatenate([v_w.astype(f32).T[C:],
                            v_b.astype(f32)[None, :]], 0)).astype(bf16),
        "x0p": pos_pad.astype(bf16),
        "x0c": pos3.astype(bf16),
        "fcw": np.ascontiguousarray(fc_w.astype(f32).T).astype(bf16),
        "fcb": np.ascontiguousarray(fc_b.astype(f32).reshape(C, 1)),
        "onesd": np.ones((128, 128), f32).astype(bf16),
    }

    in_maps = []
    for c in range(8):
        b, r = c // 2, c % 2
        xb = x[b].reshape(C, N).astype(f32)
        qcols = np.concatenate(
            [np.arange(NBLK * j, NBLK * (j + 1)) for j in BLOCKS[r]])
        in_maps.append(dict(
            shared,
            x0b=xb.astype(bf16),
            xq=np.ascontiguousarray(xb[:, qcols]).astype(bf16),
            xqp=np.ascontiguousarray(pos_pad[:, qcols]).astype(bf16),
            xres=np.ascontiguousarray(xb[:, qcols]),
            masks=masks[r],
        ))
    return in_maps


def _gather(results):
    out = np.empty((B, C, N), np.float32)
    for c in range(8):
        b, r = c // 2, c % 2
        oc = results[c]["out"]
        for s, j in enumerate(BLOCKS[r]):
            out[b][:, NBLK * j:NBLK * (j + 1)] = oc[:, NBLK * s:NBLK * (s + 1)]
    return out.reshape(B, C, S, S)


def run(trace=False, **inputs):
    from concourse import bass_utils
    global _PROGRAM
    if _PROGRAM is None:
        _PROGRAM = _build_program()
    in_maps = _host_prep(**inputs)
    res = bass_utils.run_bass_kernel_spmd(
        _PROGRAM, in_maps, list(range(8)), trace=trace)
    return _gather(res.results), res


def kernel(**inputs):
    out, _ = run(trace=False, **inputs)
    return out


# revision 26
# speedup vs baseline: 1.0413x; 1.0413x over previous
"""Trainium2 Bass kernel for the AttentionBlock problem.

Reference semantics (shapes hardcoded):
    x [4, 256, 64, 64]; 1x1-conv weights q_w/k_w/v_w [256, 258] (+biases),
    fc_w [256, 256], fc_b [256].
    x0 = concat(x, pos) -> [B, 258, 4096]
    q/k/v = relu(W @ x0 + b)                    [B, 256, 4096]
    attn  = softmax_causal(q^T k)               [B, 4096, 4096]
    out   = x + relu(fc_w @ (attn @ v^T)^T + fc_b)

Distribution: 8 cores = 4 batches x 2 query-block roles. Each core
computes full k / v^T for its batch, q only for its 4 owned 512-wide
query blocks, and causal attention for those blocks. Causal work is
balanced by giving role 0 global blocks [0,3,4,7] and role 1 blocks
[1,2,5,6]; both roles run the identical SPMD program with per-slot
key-tile counts [8,16,24,32] (slightly padded); per-core mask data
zeroes padded/non-causal entries.

The whole kernel runs in bf16 (PSUM f32): measured on this hardware,
bf16 and f32r matmuls issue at the same rate, but f32r weight loads
are 2x slower and f32r<->bf16 switches cost ~250-450ns, so all-bf16
both removes every switch and halves weight-load time.  Host-sim
error of the all-bf16 path is ~0.9e-2 vs the 2e-2 tolerance.
Softmax runs without max-subtraction (scores ~20..67, far below
overflow); denominator via a replicated ones-matmul of quad sums.
Engine placement (measured: GpSimd is pathologically slow - avoid):
ScalarE exp + k/q/fc relus, VectorE v relus/masks/quads/normalize/
residual, PE all matmuls including the rank-3 pos+bias accumulation
terms.  Input DMAs are split across the two HWDGE queues (sync +
scalar engines) and x0 is resident, transferred in per-pair column
chunks so the first projections start as early as possible.

Measured on this problem's hardware: 8 cores, ~142 us vs the 183 us
f32r/bf16 baseline; dominated by Tensor-engine busy time (~116 us at
~97% issue efficiency).  fp8e4m3 DoubleRow attn@v was implemented and
passed correctness (rel err 1.29e-2) but lost end-to-end: the
per-query rescale into fp8 costs ~1.6 us per [128,512] tile on DVE
(fp8 writes fall off the 2x path), exceeding the PE savings.
"""

import numpy as np

B = 4
C = 256
S = 64
N = S * S            # 4096
K = 256              # q/k/v channels
NBLK = 512           # query block width
NSLOT = 4            # owned query blocks per core
M_S = (8, 16, 24, 32)  # key-tile count per slot (128-wide key tiles)
BLOCKS = ((0, 3, 4, 7), (1, 2, 5, 6))  # role -> global block ids

_PROGRAM = None


def _build_program():
    import concourse.bacc as bacc
    import concourse.mybir as mybir
    import concourse.tile as tile

    F32 = mybir.dt.float32
    BF16 = mybir.dt.bfloat16
    Act = mybir.ActivationFunctionType

    nc = bacc.Bacc("TRN2", target_bir_lowering=False, debug=False)

    x0b_d = nc.dram_tensor("x0b", [C, N], BF16, kind="ExternalInput")
    x0p_d = nc.dram_tensor("x0p", [128, N], BF16, kind="ExternalInput")
    x0c_d = nc.dram_tensor("x0c", [3, N], BF16, kind="ExternalInput")
    xq_d = nc.dram_tensor("xq", [C, NSLOT * NBLK], BF16, kind="ExternalInput")
    xqp_d = nc.dram_tensor("xqp", [128, NSLOT * NBLK], BF16,
                           kind="ExternalInput")
    wq_d = nc.dram_tensor("wq", [C, K], BF16, kind="ExternalInput")
    wk_d = nc.dram_tensor("wk", [C, K], BF16, kind="ExternalInput")
    wv_d = nc.dram_tensor("wv", [C, K], BF16, kind="ExternalInput")
    wqp_d = nc.dram_tensor("wqp", [128, K], BF16, kind="ExternalInput")
    wkp_d = nc.dram_tensor("wkp", [128, K], BF16, kind="ExternalInput")
    wvc_d = nc.dram_tensor("wvc", [3, K], BF16, kind="ExternalInput")
    fcw_d = nc.dram_tensor("fcw", [C, C], BF16, kind="ExternalInput")
    fcb_d = nc.dram_tensor("fcb", [C, 1], F32, kind="ExternalInput")
    msk_d = nc.dram_tensor("masks", [NSLOT, 8, 128, NBLK], BF16,
                           kind="ExternalInput")
    od_d = nc.dram_tensor("onesd", [128, 128], BF16, kind="ExternalInput")
    xres_d = nc.dram_tensor("xres", [C, NSLOT * NBLK], F32,
                            kind="ExternalInput")
    out_d = nc.dram_tensor("out", [C, NSLOT * NBLK], F32, kind="ExternalOutput")

    with tile.TileContext(nc) as tc:
        with (
            tc.tile_pool(name="wts", bufs=1) as wts,
            tc.tile_pool(name="kqv_p", bufs=1) as kqv_p,
            tc.tile_pool(name="msk_p", bufs=6) as msk_p,
            tc.tile_pool(name="ex_p", bufs=10) as ex_p,
            tc.tile_pool(name="ds_p", bufs=4) as ds_p,
            tc.tile_pool(name="f_p", bufs=2) as f_p,
            tc.tile_pool(name="o_p", bufs=2) as o_p,
            tc.tile_pool(name="tr_p", bufs=2) as tr_p,
            tc.tile_pool(name="ps_sc", bufs=4, space="PSUM") as ps_sc,
            tc.tile_pool(name="ps_d1", bufs=1, space="PSUM") as ps_d1,
            tc.tile_pool(name="ps_o", bufs=1, space="PSUM") as ps_o,
            tc.tile_pool(name="ps_fc", bufs=1, space="PSUM") as ps_fc,
        ):
            def wtile(dram, r0, rn, dt, tag, eng=None):
                t = wts.tile([rn, dram.shape[1]], dt, tag=tag, name=tag)
                (eng or nc.sync).dma_start(t[:], dram[r0:r0 + rn, :])
                return t

            # resident x0 (+pos pad), DMAed in per-pair column chunks so
            # the first projections are not gated on the full transfer
            x0_t = [kqv_p.tile([128, N], BF16, tag=f"x0{ci}", name=f"x0{ci}")
                    for ci in range(2)]
            x0p_t = kqv_p.tile([128, N], BF16, tag="x0p", name="x0p")
            x0c_t = kqv_p.tile([3, N], BF16, tag="x0c", name="x0c")

            # phase-A-first weights (k, v) so PE can start early
            wk_t = [wtile(wk_d, 0, 128, BF16, "wk0"),
                    wtile(wk_d, 128, 128, BF16, "wk1"),
                    wtile(wkp_d, 0, 128, BF16, "wk2")]

            def chunk_dma(c0, cn):
                sl = slice(c0, c0 + cn)
                nc.sync.dma_start(x0_t[0][:, sl], x0b_d[0:128, sl])
                nc.scalar.dma_start(x0_t[1][:, sl], x0b_d[128:256, sl])
                nc.scalar.dma_start(x0p_t[:, sl], x0p_d[:, sl])
                nc.sync.dma_start(x0c_t[:, sl], x0c_d[:, sl])

            # first 512 columns only, so nb-0's k matmuls start ASAP
            chunk_dma(0, 512)
            wv_t = [wtile(wv_d, 0, 128, BF16, "wv0", eng=nc.scalar),
                    wtile(wv_d, 128, 128, BF16, "wv1", eng=nc.scalar),
                    wtile(wvc_d, 0, 3, BF16, "wv2", eng=nc.scalar)]
            chunk_dma(512, 512)

            k_sb = [[None] * 8 for _ in range(2)]
            vT_sb = [None] * 32

            def emit_pair(nbp):
                for nb in (2 * nbp, 2 * nbp + 1):
                    sl = slice(NBLK * nb, NBLK * (nb + 1))
                    for kt in range(2):
                        kts = slice(128 * kt, 128 * (kt + 1))
                        pk = ps_sc.tile([128, NBLK], F32, tag="sc",
                                        name=f"pk{kt}_{nb}")
                        nc.tensor.matmul(pk[:], wk_t[0][:, kts],
                                         x0_t[0][:, sl], start=True,
                                         stop=False)
                        nc.tensor.matmul(pk[:], wk_t[1][:, kts],
                                         x0_t[1][:, sl], start=False,
                                         stop=False)
                        nc.tensor.matmul(pk[:], wk_t[2][:, kts],
                                         x0p_t[:, sl], start=False,
                                         stop=True)
                        kt_sb = kqv_p.tile([128, NBLK], BF16,
                                           tag=f"k{kt}_{nb}",
                                           name=f"k{kt}_{nb}")
                        nc.scalar.activation(kt_sb[:], pk[:], Act.Relu)
                        k_sb[kt][nb] = kt_sb
                for nb in (2 * nbp, 2 * nbp + 1):
                    for sub in range(4):
                        i = 4 * nb + sub
                        ss = slice(128 * i, 128 * (i + 1))
                        pv = ps_sc.tile([128, K], F32, tag="sc",
                                        name=f"pv{i}")
                        nc.tensor.matmul(pv[:], x0_t[0][:, ss], wv_t[0][:],
                                         start=True, stop=False)
                        nc.tensor.matmul(pv[:], x0_t[1][:, ss], wv_t[1][:],
                                         start=False, stop=False)
                        nc.tensor.matmul(pv[:], x0c_t[:, ss], wv_t[2][:],
                                         start=False, stop=True)
                        vt_sb = kqv_p.tile([128, K], BF16, tag=f"v{i}",
                                           name=f"v{i}")
                        nc.vector.tensor_scalar_max(vt_sb[:], pv[:], 0.0)
                        vT_sb[i] = vt_sb

            q_sb = [[None] * NSLOT for _ in range(2)]

            def emit_q(s):
                sl = slice(NBLK * s, NBLK * (s + 1))
                for kt in range(2):
                    kts = slice(128 * kt, 128 * (kt + 1))
                    pq = ps_sc.tile([128, NBLK], F32, tag="sc",
                                    name=f"pq{kt}_{s}")
                    nc.tensor.matmul(pq[:], wq_t[0][:, kts], xq_t[0][:, sl],
                                     start=True, stop=False)
                    nc.tensor.matmul(pq[:], wq_t[1][:, kts], xq_t[1][:, sl],
                                     start=False, stop=False)
                    nc.tensor.matmul(pq[:], wq_t[2][:, kts], xqp_t[:, sl],
                                     start=False, stop=True)
                    qt = kqv_p.tile([128, NBLK], BF16, tag=f"q{kt}_{s}",
                                    name=f"q{kt}_{s}")
                    nc.scalar.activation(qt[:], pq[:], Act.Relu)
                    q_sb[kt][s] = qt

            slot_state = {}  # s -> (po, pd)

            def emit_slot(s, pending=None):
                """scores, exp, masks, den quads, attn@v for slot s;
                finalize(pending) is emitted after the first quad so its
                PSUM banks free early and the fc work hides in this
                slot's score stream."""
                M = M_S[s]
                po = [ps_o.tile([128, NBLK], F32, tag=f"o{vt}",
                                name=f"po{vt}_{s}") for vt in range(2)]
                pd = ps_d1.tile([128, NBLK], F32, tag="d1", name=f"pd{s}")
                ex_tiles = [None] * M
                for i in range(M):
                    psc = ps_sc.tile([128, NBLK], F32, tag="sc",
                                     name=f"psc{s}_{i}")
                    for kt in range(2):
                        nc.tensor.matmul(
                            psc[:],
                            k_sb[kt][i // 4][:, 128 * (i % 4):128 * (i % 4 + 1)],
                            q_sb[kt][s][:], start=(kt == 0), stop=(kt == 1))
                    ex = ex_p.tile([128, NBLK], BF16, tag="ex",
                                   name=f"ex{s}_{i}")
                    nc.scalar.activation(ex[:], psc[:], Act.Exp)
                    if i >= M - 8:
                        mk = msk_p.tile([128, NBLK], BF16, tag="mk",
                                        name=f"mk{s}_{i}")
                        nc.scalar.dma_start(mk[:], msk_d[s, i - (M - 8)])
                        nc.vector.tensor_mul(ex[:], ex[:], mk[:])
                    ex_tiles[i] = ex
                    if i % 4 == 3:
                        j = i - 3
                        if j == 0 and pending is not None:
                            finalize(pending)
                        # attn@v for the finished quad
                        for jj in range(j, j + 4):
                            e = ex_tiles[jj]
                            for vt in range(2):
                                nc.tensor.matmul(
                                    po[vt][:],
                                    vT_sb[jj][:, 128 * vt:128 * (vt + 1)],
                                    e[:], start=(jj == 0), stop=(jj == M - 1))
                        # quad-summed denominator
                        da = ds_p.tile([128, NBLK], BF16, tag="ds",
                                       name=f"da{s}_{j}")
                        nc.vector.tensor_add(da[:], ex_tiles[j][:],
                                             ex_tiles[j + 1][:])
                        db = ds_p.tile([128, NBLK], BF16, tag="ds",
                                       name=f"db{s}_{j}")
                        nc.vector.tensor_add(db[:], ex_tiles[j + 2][:],
                                             ex_tiles[j + 3][:])
                        dsum = ds_p.tile([128, NBLK], BF16, tag="ds",
                                         name=f"ds{s}_{j}")
                        nc.vector.tensor_add(dsum[:], da[:], db[:])
                        nc.tensor.matmul(pd[:], od_t[:], dsum[:],
                                         start=(j == 0), stop=(j == M - 4))
                        for jj in range(j, j + 4):
                            ex_tiles[jj] = None
                slot_state[s] = (po, pd)

            def finalize(s, halves=1):
                """normalize, fc, relu, residual, dma out for slot s.
                halves=2 pipelines the chain in 256-col pieces (used on
                the last slot, where this chain is the kernel tail)."""
                po, pd = slot_state[s]
                hw = NBLK // halves
                for h in range(halves):
                    hs = slice(h * hw, (h + 1) * hw)
                    rb = f_p.tile([128, hw], F32, tag="rb",
                                  name=f"rb{s}_{h}")
                    nc.vector.reciprocal_approx_fast(rb[:], pd[:, hs])
                    o_sb = []
                    for vt in range(2):
                        ot = o_p.tile([128, hw], BF16, tag=f"ob{vt}",
                                      name=f"ob{vt}_{s}_{h}")
                        nc.vector.tensor_mul(ot[:], po[vt][:, hs], rb[:])
                        o_sb.append(ot)
                    for ot in range(2):
                        pfc = ps_fc.tile([128, hw], F32, tag="fc",
                                         name=f"pfc{ot}_{s}_{h}")
                        for vt in range(2):
                            nc.tensor.matmul(
                                pfc[:],
                                fcw_t[vt][:, 128 * ot:128 * (ot + 1)],
                                o_sb[vt][:], start=(vt == 0),
                                stop=(vt == 1))
                        t_sb = tr_p.tile([128, hw], F32, tag=f"t{ot}",
                                         name=f"t{ot}_{s}_{h}")
                        nc.scalar.activation(t_sb[:], pfc[:], Act.Relu,
                                             bias=fcb_t[ot][:])
                        r_sb = tr_p.tile([128, hw], F32, tag=f"r{ot}",
                                         name=f"r{ot}_{s}_{h}")
                        nc.vector.tensor_add(
                            r_sb[:], t_sb[:],
                            xres_t[ot][:, NBLK * s + h * hw:
                                        NBLK * s + (h + 1) * hw])
                        nc.sync.dma_start(
                            out_d[128 * ot:128 * (ot + 1),
                                  NBLK * s + h * hw:
                                  NBLK * s + (h + 1) * hw], r_sb[:])

            # ---- emission schedule: spread k/v pairs between slots so
            # DMA and projections overlap attention; all bf16, no dtype
            # switches anywhere.
            emit_pair(0)

            # q inputs next on the queues (q runs right after pair 0) ...
            wq_t = [wtile(wq_d, 0, 128, BF16, "wq0"),
                    wtile(wq_d, 128, 128, BF16, "wq1"),
                    wtile(wqp_d, 0, 128, BF16, "wq2")]
            xq_t = [wtile(xq_d, 0, 128, BF16, "xq0"),
                    wtile(xq_d, 128, 128, BF16, "xq1")]
            xqp_t = wtile(xqp_d, 0, 128, BF16, "xqp")
            fcb_t = [wtile(fcb_d, 0, 128, F32, "fcb0"),
                     wtile(fcb_d, 128, 128, F32, "fcb1")]
            od_t = wtile(od_d, 0, 128, BF16, "onesd")

            emit_q(0)
            emit_slot(0)
            chunk_dma(1024, 1024)
            fcw_t = [wtile(fcw_d, 0, 128, BF16, "fcw0", eng=nc.scalar),
                     wtile(fcw_d, 128, 128, BF16, "fcw1", eng=nc.scalar)]
            xres_t = [wtile(xres_d, 128 * ot, 128, F32, f"xres{ot}",
                            eng=nc.scalar) for ot in range(2)]
            emit_q(1)
            emit_pair(1)
            emit_slot(1, pending=0)
            chunk_dma(2048, 1024)
            emit_q(2)
            emit_pair(2)
            emit_slot(2, pending=1)
            chunk_dma(3072, 1024)
            emit_q(3)
            emit_pair(3)
            emit_slot(3, pending=2)
            finalize(3)

    nc.compile()
    return nc


def _host_prep(x, q_w, q_b, k_w, k_b, v_w, v_b, fc_w, fc_b):
    """Build the per-core input maps."""
    import ml_dtypes
    f32 = np.float32
    bf16 = ml_dtypes.bfloat16
    n = np.arange(N)
    px = ((n // S) / S).astype(f32)
    py = ((n % S) / S).astype(f32)
    pos3 = np.stack([px, py, np.ones(N, f32)])   # [3, N] incl bias channel

    pos_pad = np.zeros((128, N), f32)
    pos_pad[:3] = pos3

    def pad_w(w, b):
        # rows 0..1 = pos weight rows, row 2 = bias, rest zero
        p = np.zeros((128, K), f32)
        p[:2] = w.astype(f32).T[C:]
        p[2] = b.astype(f32)
        return p

    # per-role masks [NSLOT, 8, 128, 512]
    mm = np.arange(128)[:, None]
    nn = np.arange(NBLK)[None, :]
    masks = {}
    for r in range(2):
        mr = np.zeros((NSLOT, 8, 128, NBLK), f32)
        for s in range(NSLOT):
            j = BLOCKS[r][s]
            for t in range(8):
                i = M_S[s] - 8 + t
                mr[s, t] = (128 * i + mm <= 512 * j + nn)
        masks[r] = mr.astype(bf16)

    shared = {
        "wq": np.ascontiguousarray(q_w.astype(f32).T[:C]).astype(bf16),
        "wk": np.ascontiguousarray(k_w.astype(f32).T[:C]).astype(bf16),
        "wv": np.ascontiguousarray(v_w.astype(f32).T[:C]).astype(bf16),
        "wqp": pad_w(q_w, q_b).astype(bf16),
        "wkp": pad_w(k_w, k_b).astype(bf16),
        "wvc": np.ascontiguousarray(
            np.concatenate([v_w.astype(f32).T[C:],
                            v_b.astype(f32)[None, :]], 0)).astype(bf16),
        "x0p": pos_pad.astype(bf16),
        "x0c": pos3.astype(bf16),
        "fcw": np.ascontiguousarray(fc_w.astype(f32).T).astype(bf16),
        "fcb": np.ascontiguousarray(fc_b.astype(f32).reshape(C, 1)),
        "onesd": np.ones((128, 128), f32).astype(bf16),
    }

    in_maps = []
    for c in range(8):
        b, r = c // 2, c % 2
        xb = x[b].reshape(C, N).astype(f32)
        qcols = np.concatenate(
            [np.arange(NBLK * j, NBLK * (j + 1)) for j in BLOCKS[r]])
        in_maps.append(dict(
            shared,
            x0b=xb.astype(bf16),
            xq=np.ascontiguousarray(xb[:, qcols]).astype(bf16),
            xqp=np.ascontiguousarray(pos_pad[:, qcols]).astype(bf16),
            xres=np.ascontiguousarray(xb[:, qcols]),
            masks=masks[r],
        ))
    return in_maps


def _gather(results):
    out = np.empty((B, C, N), np.float32)
    for c in range(8):
        b, r = c // 2, c % 2
        oc = results[c]["out"]
        for s, j in enumerate(BLOCKS[r]):
            out[b][:, NBLK * j:NBLK * (j + 1)] = oc[:, NBLK * s:NBLK * (s + 1)]
    return out.reshape(B, C, S, S)


def run(trace=False, **inputs):
    from concourse import bass_utils
    global _PROGRAM
    if _PROGRAM is None:
        _PROGRAM = _build_program()
    in_maps = _host_prep(**inputs)
    res = bass_utils.run_bass_kernel_spmd(
        _PROGRAM, in_maps, list(range(8)), trace=trace)
    return _gather(res.results), res


def kernel(**inputs):
    out, _ = run(trace=False, **inputs)
    return out


# revision 27
# speedup vs baseline: 1.0441x; 1.0027x over previous
"""Trainium2 Bass kernel for the AttentionBlock problem.

Reference semantics (shapes hardcoded):
    x [4, 256, 64, 64]; 1x1-conv weights q_w/k_w/v_w [256, 258] (+biases),
    fc_w [256, 256], fc_b [256].
    x0 = concat(x, pos) -> [B, 258, 4096]
    q/k/v = relu(W @ x0 + b)                    [B, 256, 4096]
    attn  = softmax_causal(q^T k)               [B, 4096, 4096]
    out   = x + relu(fc_w @ (attn @ v^T)^T + fc_b)

Distribution: 8 cores = 4 batches x 2 query-block roles. Each core
computes full k / v^T for its batch, q only for its 4 owned 512-wide
query blocks, and causal attention for those blocks. Causal work is
balanced by giving role 0 global blocks [0,3,4,7] and role 1 blocks
[1,2,5,6]; both roles run the identical SPMD program with per-slot
key-tile counts [8,16,24,32] (slightly padded); per-core mask data
zeroes padded/non-causal entries.

The whole kernel runs in bf16 (PSUM f32): measured on this hardware,
bf16 and f32r matmuls issue at the same rate, but f32r weight loads
are 2x slower and f32r<->bf16 switches cost ~250-450ns, so all-bf16
both removes every switch and halves weight-load time.  Host-sim
error of the all-bf16 path is ~0.9e-2 vs the 2e-2 tolerance.
Softmax runs without max-subtraction (scores ~20..67, far below
overflow); denominator via a replicated ones-matmul of quad sums.
Engine placement (measured: GpSimd is pathologically slow - avoid):
ScalarE exp + k/q/fc relus, VectorE v relus/masks/quads/normalize/
residual, PE all matmuls including the rank-3 pos+bias accumulation
terms.  Input DMAs are split across the two HWDGE queues (sync +
scalar engines) and x0 is resident, transferred in per-pair column
chunks so the first projections start as early as possible.

Measured on this problem's hardware: 8 cores, ~142 us (best 141.5,
run-to-run noise ~+/-3 us) vs the 183 us f32r/bf16 baseline;
dominated by Tensor-engine busy time (~116 us at ~97% issue
efficiency), i.e. this is at the bf16-matmul floor of the
decomposition.  Dead ends validated on hardware: fp8e4m3 DoubleRow
attn@v passed correctness (1.29e-2) but the per-query fp8 rescale
costs ~1.6 us per [128,512] tile on DVE (fp8 writes fall off the 2x
path), exceeding the PE savings; fp8 scores fail numerics (7.4e-2);
DMA-to-PSUM pos preload is rejected by the bass API; GpSimd
elementwise ops run ~5-40x slower than spec; the 80-vs-72 key-tile
role padding is provably irreducible under the same-program SPMD
constraint.
"""

import numpy as np

B = 4
C = 256
S = 64
N = S * S            # 4096
K = 256              # q/k/v channels
NBLK = 512           # query block width
NSLOT = 4            # owned query blocks per core
M_S = (8, 16, 24, 32)  # key-tile count per slot (128-wide key tiles)
BLOCKS = ((0, 3, 4, 7), (1, 2, 5, 6))  # role -> global block ids

_PROGRAM = None


def _build_program():
    import concourse.bacc as bacc
    import concourse.mybir as mybir
    import concourse.tile as tile

    F32 = mybir.dt.float32
    BF16 = mybir.dt.bfloat16
    Act = mybir.ActivationFunctionType

    nc = bacc.Bacc("TRN2", target_bir_lowering=False, debug=False)

    x0b_d = nc.dram_tensor("x0b", [C, N], BF16, kind="ExternalInput")
    x0p_d = nc.dram_tensor("x0p", [128, N], BF16, kind="ExternalInput")
    x0c_d = nc.dram_tensor("x0c", [3, N], BF16, kind="ExternalInput")
    xq_d = nc.dram_tensor("xq", [C, NSLOT * NBLK], BF16, kind="ExternalInput")
    xqp_d = nc.dram_tensor("xqp", [128, NSLOT * NBLK], BF16,
                           kind="ExternalInput")
    wq_d = nc.dram_tensor("wq", [C, K], BF16, kind="ExternalInput")
    wk_d = nc.dram_tensor("wk", [C, K], BF16, kind="ExternalInput")
    wv_d = nc.dram_tensor("wv", [C, K], BF16, kind="ExternalInput")
    wqp_d = nc.dram_tensor("wqp", [128, K], BF16, kind="ExternalInput")
    wkp_d = nc.dram_tensor("wkp", [128, K], BF16, kind="ExternalInput")
    wvc_d = nc.dram_tensor("wvc", [3, K], BF16, kind="ExternalInput")
    fcw_d = nc.dram_tensor("fcw", [C, C], BF16, kind="ExternalInput")
    fcb_d = nc.dram_tensor("fcb", [C, 1], F32, kind="ExternalInput")
    msk_d = nc.dram_tensor("masks", [NSLOT, 8, 128, NBLK], BF16,
                           kind="ExternalInput")
    od_d = nc.dram_tensor("onesd", [128, 128], BF16, kind="ExternalInput")
    xres_d = nc.dram_tensor("xres", [C, NSLOT * NBLK], F32,
                            kind="ExternalInput")
    out_d = nc.dram_tensor("out", [C, NSLOT * NBLK], F32, kind="ExternalOutput")

    with tile.TileContext(nc) as tc:
        with (
            tc.tile_pool(name="wts", bufs=1) as wts,
            tc.tile_pool(name="kqv_p", bufs=1) as kqv_p,
            tc.tile_pool(name="msk_p", bufs=6) as msk_p,
            tc.tile_pool(name="ex_p", bufs=10) as ex_p,
            tc.tile_pool(name="ds_p", bufs=4) as ds_p,
            tc.tile_pool(name="f_p", bufs=2) as f_p,
            tc.tile_pool(name="o_p", bufs=2) as o_p,
            tc.tile_pool(name="tr_p", bufs=2) as tr_p,
            tc.tile_pool(name="ps_sc", bufs=4, space="PSUM") as ps_sc,
            tc.tile_pool(name="ps_d1", bufs=1, space="PSUM") as ps_d1,
            tc.tile_pool(name="ps_o", bufs=1, space="PSUM") as ps_o,
            tc.tile_pool(name="ps_fc", bufs=1, space="PSUM") as ps_fc,
        ):
            def wtile(dram, r0, rn, dt, tag, eng=None):
                t = wts.tile([rn, dram.shape[1]], dt, tag=tag, name=tag)
                (eng or nc.sync).dma_start(t[:], dram[r0:r0 + rn, :])
                return t

            # resident x0 (+pos pad), DMAed in per-pair column chunks so
            # the first projections are not gated on the full transfer
            x0_t = [kqv_p.tile([128, N], BF16, tag=f"x0{ci}", name=f"x0{ci}")
                    for ci in range(2)]
            x0p_t = kqv_p.tile([128, N], BF16, tag="x0p", name="x0p")
            x0c_t = kqv_p.tile([3, N], BF16, tag="x0c", name="x0c")

            # phase-A-first weights (k, v) so PE can start early
            wk_t = [wtile(wk_d, 0, 128, BF16, "wk0"),
                    wtile(wk_d, 128, 128, BF16, "wk1"),
                    wtile(wkp_d, 0, 128, BF16, "wk2")]

            def chunk_dma(c0, cn):
                sl = slice(c0, c0 + cn)
                nc.sync.dma_start(x0_t[0][:, sl], x0b_d[0:128, sl])
                nc.scalar.dma_start(x0_t[1][:, sl], x0b_d[128:256, sl])
                nc.scalar.dma_start(x0p_t[:, sl], x0p_d[:, sl])
                nc.sync.dma_start(x0c_t[:, sl], x0c_d[:, sl])

            # first 512 columns only, so nb-0's k matmuls start ASAP
            chunk_dma(0, 512)
            wv_t = [wtile(wv_d, 0, 128, BF16, "wv0", eng=nc.scalar),
                    wtile(wv_d, 128, 128, BF16, "wv1", eng=nc.scalar),
                    wtile(wvc_d, 0, 3, BF16, "wv2", eng=nc.scalar)]
            chunk_dma(512, 512)

            k_sb = [[None] * 8 for _ in range(2)]
            vT_sb = [None] * 32

            def emit_pair(nbp):
                for nb in (2 * nbp, 2 * nbp + 1):
                    sl = slice(NBLK * nb, NBLK * (nb + 1))
                    for kt in range(2):
                        kts = slice(128 * kt, 128 * (kt + 1))
                        pk = ps_sc.tile([128, NBLK], F32, tag="sc",
                                        name=f"pk{kt}_{nb}")
                        nc.tensor.matmul(pk[:], wk_t[0][:, kts],
                                         x0_t[0][:, sl], start=True,
                                         stop=False)
                        nc.tensor.matmul(pk[:], wk_t[1][:, kts],
                                         x0_t[1][:, sl], start=False,
                                         stop=False)
                        nc.tensor.matmul(pk[:], wk_t[2][:, kts],
                                         x0p_t[:, sl], start=False,
                                         stop=True)
                        kt_sb = kqv_p.tile([128, NBLK], BF16,
                                           tag=f"k{kt}_{nb}",
                                           name=f"k{kt}_{nb}")
                        nc.scalar.activation(kt_sb[:], pk[:], Act.Relu)
                        k_sb[kt][nb] = kt_sb
                for nb in (2 * nbp, 2 * nbp + 1):
                    for sub in range(4):
                        i = 4 * nb + sub
                        ss = slice(128 * i, 128 * (i + 1))
                        pv = ps_sc.tile([128, K], F32, tag="sc",
                                        name=f"pv{i}")
                        nc.tensor.matmul(pv[:], x0_t[0][:, ss], wv_t[0][:],
                                         start=True, stop=False)
                        nc.tensor.matmul(pv[:], x0_t[1][:, ss], wv_t[1][:],
                                         start=False, stop=False)
                        nc.tensor.matmul(pv[:], x0c_t[:, ss], wv_t[2][:],
                                         start=False, stop=True)
                        vt_sb = kqv_p.tile([128, K], BF16, tag=f"v{i}",
                                           name=f"v{i}")
                        nc.vector.tensor_scalar_max(vt_sb[:], pv[:], 0.0)
                        vT_sb[i] = vt_sb

            q_sb = [[None] * NSLOT for _ in range(2)]

            def emit_q(s):
                sl = slice(NBLK * s, NBLK * (s + 1))
                for kt in range(2):
                    kts = slice(128 * kt, 128 * (kt + 1))
                    pq = ps_sc.tile([128, NBLK], F32, tag="sc",
                                    name=f"pq{kt}_{s}")
                    nc.tensor.matmul(pq[:], wq_t[0][:, kts], xq_t[0][:, sl],
                                     start=True, stop=False)
                    nc.tensor.matmul(pq[:], wq_t[1][:, kts], xq_t[1][:, sl],
                                     start=False, stop=False)
                    nc.tensor.matmul(pq[:], wq_t[2][:, kts], xqp_t[:, sl],
                                     start=False, stop=True)
                    qt = kqv_p.tile([128, NBLK], BF16, tag=f"q{kt}_{s}",
                                    name=f"q{kt}_{s}")
                    nc.scalar.activation(qt[:], pq[:], Act.Relu)
                    q_sb[kt][s] = qt

            slot_state = {}  # s -> (po, pd)

            def emit_slot(s, pending=None):
                """scores, exp, masks, den quads, attn@v for slot s;
                finalize(pending) is emitted after the first quad so its
                PSUM banks free early and the fc work hides in this
                slot's score stream."""
                M = M_S[s]
                po = [ps_o.tile([128, NBLK], F32, tag=f"o{vt}",
                                name=f"po{vt}_{s}") for vt in range(2)]
                pd = ps_d1.tile([128, NBLK], F32, tag="d1", name=f"pd{s}")
                ex_tiles = [None] * M
                for i in range(M):
                    psc = ps_sc.tile([128, NBLK], F32, tag="sc",
                                     name=f"psc{s}_{i}")
                    for kt in range(2):
                        nc.tensor.matmul(
                            psc[:],
                            k_sb[kt][i // 4][:, 128 * (i % 4):128 * (i % 4 + 1)],
                            q_sb[kt][s][:], start=(kt == 0), stop=(kt == 1))
                    ex = ex_p.tile([128, NBLK], BF16, tag="ex",
                                   name=f"ex{s}_{i}")
                    nc.scalar.activation(ex[:], psc[:], Act.Exp)
                    if i >= M - 8:
                        mk = msk_p.tile([128, NBLK], BF16, tag="mk",
                                        name=f"mk{s}_{i}")
                        nc.scalar.dma_start(mk[:], msk_d[s, i - (M - 8)])
                        nc.vector.tensor_mul(ex[:], ex[:], mk[:])
                    ex_tiles[i] = ex
                    if i % 4 == 3:
                        j = i - 3
                        if j == 0 and pending is not None:
                            finalize(pending)
                        # attn@v for the finished quad
                        for jj in range(j, j + 4):
                            e = ex_tiles[jj]
                            for vt in range(2):
                                nc.tensor.matmul(
                                    po[vt][:],
                                    vT_sb[jj][:, 128 * vt:128 * (vt + 1)],
                                    e[:], start=(jj == 0), stop=(jj == M - 1))
                        # quad-summed denominator
                        da = ds_p.tile([128, NBLK], BF16, tag="ds",
                                       name=f"da{s}_{j}")
                        nc.vector.tensor_add(da[:], ex_tiles[j][:],
                                             ex_tiles[j + 1][:])
                        db = ds_p.tile([128, NBLK], BF16, tag="ds",
                                       name=f"db{s}_{j}")
                        nc.vector.tensor_add(db[:], ex_tiles[j + 2][:],
                                             ex_tiles[j + 3][:])
                        dsum = ds_p.tile([128, NBLK], BF16, tag="ds",
                                         name=f"ds{s}_{j}")
                        nc.vector.tensor_add(dsum[:], da[:], db[:])
                        nc.tensor.matmul(pd[:], od_t[:], dsum[:],
                                         start=(j == 0), stop=(j == M - 4))
                        for jj in range(j, j + 4):
                            ex_tiles[jj] = None
                slot_state[s] = (po, pd)

            def finalize(s, halves=1):
                """normalize, fc, relu, residual, dma out for slot s.
                halves=2 pipelines the chain in 256-col pieces (used on
                the last slot, where this chain is the kernel tail)."""
                po, pd = slot_state[s]
                hw = NBLK // halves
                for h in range(halves):
                    hs = slice(h * hw, (h + 1) * hw)
                    rb = f_p.tile([128, hw], F32, tag="rb",
                                  name=f"rb{s}_{h}")
                    nc.vector.reciprocal_approx_fast(rb[:], pd[:, hs])
                    o_sb = []
                    for vt in range(2):
                        ot = o_p.tile([128, hw], BF16, tag=f"ob{vt}",
                                      name=f"ob{vt}_{s}_{h}")
                        nc.vector.tensor_mul(ot[:], po[vt][:, hs], rb[:])
                        o_sb.append(ot)
                    for ot in range(2):
                        pfc = ps_fc.tile([128, hw], F32, tag="fc",
                                         name=f"pfc{ot}_{s}_{h}")
                        for vt in range(2):
                            nc.tensor.matmul(
                                pfc[:],
                                fcw_t[vt][:, 128 * ot:128 * (ot + 1)],
                                o_sb[vt][:], start=(vt == 0),
                                stop=(vt == 1))
                        t_sb = tr_p.tile([128, hw], F32, tag=f"t{ot}",
                                         name=f"t{ot}_{s}_{h}")
                        nc.scalar.activation(t_sb[:], pfc[:], Act.Relu,
                                             bias=fcb_t[ot][:])
                        r_sb = tr_p.tile([128, hw], F32, tag=f"r{ot}",
                                         name=f"r{ot}_{s}_{h}")
                        nc.vector.tensor_add(
                            r_sb[:], t_sb[:],
                            xres_t[ot][:, NBLK * s + h * hw:
                                        NBLK * s + (h + 1) * hw])
                        nc.sync.dma_start(
                            out_d[128 * ot:128 * (ot + 1),
                                  NBLK * s + h * hw:
                                  NBLK * s + (h + 1) * hw], r_sb[:])

            # ---- emission schedule: spread k/v pairs between slots so
            # DMA and projections overlap attention; all bf16, no dtype
            # switches anywhere.
            emit_pair(0)

            # q inputs next on the queues (q runs right after pair 0) ...
            wq_t = [wtile(wq_d, 0, 128, BF16, "wq0"),
                    wtile(wq_d, 128, 128, BF16, "wq1"),
                    wtile(wqp_d, 0, 128, BF16, "wq2")]
            xq_t = [wtile(xq_d, 0, 128, BF16, "xq0"),
                    wtile(xq_d, 128, 128, BF16, "xq1")]
            xqp_t = wtile(xqp_d, 0, 128, BF16, "xqp")
            fcb_t = [wtile(fcb_d, 0, 128, F32, "fcb0"),
                     wtile(fcb_d, 128, 128, F32, "fcb1")]
            od_t = wtile(od_d, 0, 128, BF16, "onesd")

            emit_q(0)
            emit_slot(0)
            chunk_dma(1024, 1024)
            fcw_t = [wtile(fcw_d, 0, 128, BF16, "fcw0", eng=nc.scalar),
                     wtile(fcw_d, 128, 128, BF16, "fcw1", eng=nc.scalar)]
            xres_t = [wtile(xres_d, 128 * ot, 128, F32, f"xres{ot}",
                            eng=nc.scalar) for ot in range(2)]
            emit_q(1)
            emit_pair(1)
            emit_slot(1, pending=0)
            chunk_dma(2048, 1024)
            emit_q(2)
            emit_pair(2)
            emit_slot(2, pending=1)
            chunk_dma(3072, 1024)
            emit_q(3)
            emit_pair(3)
            emit_slot(3, pending=2)
            finalize(3)

    nc.compile()
    return nc


def _host_prep(x, q_w, q_b, k_w, k_b, v_w, v_b, fc_w, fc_b):
    """Build the per-core input maps."""
    import ml_dtypes
    f32 = np.float32
    bf16 = ml_dtypes.bfloat16
    n = np.arange(N)
    px = ((n // S) / S).astype(f32)
    py = ((n % S) / S).astype(f32)
    pos3 = np.stack([px, py, np.ones(N, f32)])   # [3, N] incl bias channel

    pos_pad = np.zeros((128, N), f32)
    pos_pad[:3] = pos3

    def pad_w(w, b):
        # rows 0..1 = pos weight rows, row 2 = bias, rest zero
        p = np.zeros((128, K), f32)
        p[:2] = w.astype(f32).T[C:]
        p[2] = b.astype(f32)
        return p

    # per-role masks [NSLOT, 8, 128, 512]
    mm = np.arange(128)[:, None]
    nn = np.arange(NBLK)[None, :]
    masks = {}
    for r in range(2):
        mr = np.zeros((NSLOT, 8, 128, NBLK), f32)
        for s in range(NSLOT):
            j = BLOCKS[r][s]
            for t in range(8):
                i = M_S[s] - 8 + t
                mr[s, t] = (128 * i + mm <= 512 * j + nn)
        masks[r] = mr.astype(bf16)

    shared = {
        "wq": np.ascontiguousarray(q_w.astype(f32).T[:C]).astype(bf16),
        "wk": np.ascontiguousarray(k_w.astype(f32).T[:C]).astype(bf16),
        "wv": np.ascontiguousarray(v_w.astype(f32).T[:C]).astype(bf16),
        "wqp": pad_w(q_w, q_b).astype(bf16),
        "wkp": pad_w(k_w, k_b).astype(bf16),
        "wvc": np.ascontiguousarray(
            np.concatenate([v_w.astype(f32).T[C:],
                            v_b.astype(f32)[None, :]], 0)).astype(bf16),
        "x0p": pos_pad.astype(bf16),
        "x0c": pos3.astype(bf16),
        "fcw": np.ascontiguousarray(fc_w.astype(f32).T).astype(bf16),
        "fcb": np.ascontiguousarray(fc_b.astype(f32).reshape(C, 1)),
        "onesd": np.ones((128, 128), f32).astype(bf16),
    }

    in_maps = []
    for c in range(8):
        b, r = c // 2, c % 2
        xb = x[b].reshape(C, N).astype(f32)
        qcols = np.concatenate(
            [np.arange(NBLK * j, NBLK * (j + 1)) for j in BLOCKS[r]])
        in_maps.append(dict(
            shared,
            x0b=xb.astype(bf16),
            xq=np.ascontiguousarray(xb[:, qcols]).astype(bf16),
            xqp=np.ascontiguousarray(pos_pad[:, qcols]).astype(bf16),
            xres=np.ascontiguousarray(xb[:, qcols]),
            masks=masks[r],
        ))
    return in_maps


def _gather(results):
    out = np.empty((B, C, N), np.float32)
    for c in range(8):
        b, r = c // 2, c % 2
        oc = results[c]["out"]
        for s, j in enumerate(BLOCKS[r]):
            out[b][:, NBLK * j:NBLK * (j + 1)] = oc[:, NBLK * s:NBLK * (s + 1)]
    return out.reshape(B, C, S, S)


def run(trace=False, **inputs):
    from concourse import bass_utils
    global _PROGRAM
    if _PROGRAM is None:
        _PROGRAM = _build_program()
    in_maps = _host_prep(**inputs)
    res = bass_utils.run_bass_kernel_spmd(
        _PROGRAM, in_maps, list(range(8)), trace=trace)
    return _gather(res.results), res


def kernel(**inputs):
    out, _ = run(trace=False, **inputs)
    return out


# revision 28
# speedup vs baseline: 1.0607x; 1.0159x over previous
"""Trainium2 Bass kernel for the AttentionBlock problem.

Reference semantics (shapes hardcoded):
    x [4, 256, 64, 64]; 1x1-conv weights q_w/k_w/v_w [256, 258] (+biases),
    fc_w [256, 256], fc_b [256].
    x0 = concat(x, pos) -> [B, 258, 4096]
    q/k/v = relu(W @ x0 + b)                    [B, 256, 4096]
    attn  = softmax_causal(q^T k)               [B, 4096, 4096]
    out   = x + relu(fc_w @ (attn @ v^T)^T + fc_b)

Distribution: 8 cores = 4 batches x 2 query-block roles. Each core
computes full k / v^T for its batch, q only for its 4 owned 512-wide
query blocks, and causal attention for those blocks. Causal work is
balanced by giving role 0 global blocks [0,3,4,7] and role 1 blocks
[1,2,5,6]; both roles run the identical SPMD program with per-slot
key-tile counts [8,16,24,32] (slightly padded); per-core mask data
zeroes padded/non-causal entries.

The whole kernel runs in bf16 (PSUM f32): measured on this hardware,
bf16 and f32r matmuls issue at the same rate, but f32r weight loads
are 2x slower and f32r<->bf16 switches cost ~250-450ns, so all-bf16
both removes every switch and halves weight-load time.  Host-sim
error of the all-bf16 path is ~0.9e-2 vs the 2e-2 tolerance.
Softmax runs without max-subtraction (scores ~20..67, far below
overflow); denominator via a replicated ones-matmul of quad sums.
Engine placement (measured: GpSimd is pathologically slow - avoid):
ScalarE exp + k/q/fc relus, VectorE v relus/masks/quads/normalize/
residual, PE all matmuls including the rank-3 pos+bias accumulation
terms.  Input DMAs are split across the two HWDGE queues (sync +
scalar engines) and x0 is resident, transferred in per-pair column
chunks so the first projections start as early as possible.

Measured on this problem's hardware: 8 cores, ~142 us (best 141.5,
run-to-run noise ~+/-3 us) vs the 183 us f32r/bf16 baseline;
dominated by Tensor-engine busy time (~116 us at ~97% issue
efficiency), i.e. this is at the bf16-matmul floor of the
decomposition.  Dead ends validated on hardware: fp8e4m3 DoubleRow
attn@v passed correctness (1.29e-2) but the per-query fp8 rescale
costs ~1.6 us per [128,512] tile on DVE (fp8 writes fall off the 2x
path), exceeding the PE savings; fp8 scores fail numerics (7.4e-2);
DMA-to-PSUM pos preload is rejected by the bass API; GpSimd
elementwise ops run ~5-40x slower than spec; the 80-vs-72 key-tile
role padding is provably irreducible under the same-program SPMD
constraint.
"""

import numpy as np

B = 4
C = 256
S = 64
N = S * S            # 4096
K = 256              # q/k/v channels
NBLK = 512           # query block width
NSLOT = 4            # owned query blocks per core
M_S = (8, 16, 24, 32)  # key-tile count per slot (128-wide key tiles)
BLOCKS = ((0, 3, 4, 7), (1, 2, 5, 6))  # role -> global block ids

_PROGRAM = None


def _build_program():
    import concourse.bacc as bacc
    import concourse.mybir as mybir
    import concourse.tile as tile

    F32 = mybir.dt.float32
    BF16 = mybir.dt.bfloat16
    Act = mybir.ActivationFunctionType

    nc = bacc.Bacc("TRN2", target_bir_lowering=False, debug=False)

    x0b_d = nc.dram_tensor("x0b", [C, N], BF16, kind="ExternalInput")
    x0p_d = nc.dram_tensor("x0p", [128, N], BF16, kind="ExternalInput")
    x0c_d = nc.dram_tensor("x0c", [3, N], BF16, kind="ExternalInput")
    xq_d = nc.dram_tensor("xq", [C, NSLOT * NBLK], BF16, kind="ExternalInput")
    xqp_d = nc.dram_tensor("xqp", [128, NSLOT * NBLK], BF16,
                           kind="ExternalInput")
    wq_d = nc.dram_tensor("wq", [C, K], BF16, kind="ExternalInput")
    wk_d = nc.dram_tensor("wk", [C, K], BF16, kind="ExternalInput")
    wv_d = nc.dram_tensor("wv", [C, K], BF16, kind="ExternalInput")
    wqp_d = nc.dram_tensor("wqp", [128, K], BF16, kind="ExternalInput")
    wkp_d = nc.dram_tensor("wkp", [128, K], BF16, kind="ExternalInput")
    wvc_d = nc.dram_tensor("wvc", [3, K], BF16, kind="ExternalInput")
    fcw_d = nc.dram_tensor("fcw", [C, C], BF16, kind="ExternalInput")
    fcb_d = nc.dram_tensor("fcb", [C, 1], F32, kind="ExternalInput")
    msk_d = nc.dram_tensor("masks", [NSLOT, 8, 128, NBLK], BF16,
                           kind="ExternalInput")
    od_d = nc.dram_tensor("onesd", [128, 128], BF16, kind="ExternalInput")
    xres_d = nc.dram_tensor("xres", [C, NSLOT * NBLK], F32,
                            kind="ExternalInput")
    out_d = nc.dram_tensor("out", [C, NSLOT * NBLK], F32, kind="ExternalOutput")

    with tile.TileContext(nc) as tc:
        with (
            tc.tile_pool(name="wts", bufs=1) as wts,
            tc.tile_pool(name="kqv_p", bufs=1) as kqv_p,
            tc.tile_pool(name="msk_p", bufs=8) as msk_p,
            tc.tile_pool(name="ex_p", bufs=10) as ex_p,
            tc.tile_pool(name="ds_p", bufs=4) as ds_p,
            tc.tile_pool(name="f_p", bufs=2) as f_p,
            tc.tile_pool(name="o_p", bufs=2) as o_p,
            tc.tile_pool(name="tr_p", bufs=2) as tr_p,
            tc.tile_pool(name="ps_sc", bufs=4, space="PSUM") as ps_sc,
            tc.tile_pool(name="ps_d1", bufs=1, space="PSUM") as ps_d1,
            tc.tile_pool(name="ps_o", bufs=1, space="PSUM") as ps_o,
            tc.tile_pool(name="ps_fc", bufs=1, space="PSUM") as ps_fc,
        ):
            def wtile(dram, r0, rn, dt, tag, eng=None):
                t = wts.tile([rn, dram.shape[1]], dt, tag=tag, name=tag)
                (eng or nc.sync).dma_start(t[:], dram[r0:r0 + rn, :])
                return t

            # resident x0 (+pos pad), DMAed in per-pair column chunks so
            # the first projections are not gated on the full transfer
            x0_t = [kqv_p.tile([128, N], BF16, tag=f"x0{ci}", name=f"x0{ci}")
                    for ci in range(2)]
            x0p_t = kqv_p.tile([128, N], BF16, tag="x0p", name="x0p")
            x0c_t = kqv_p.tile([3, N], BF16, tag="x0c", name="x0c")

            def chunk_dma(c0, cn):
                sl = slice(c0, c0 + cn)
                nc.sync.dma_start(x0_t[0][:, sl], x0b_d[0:128, sl])
                nc.scalar.dma_start(x0_t[1][:, sl], x0b_d[128:256, sl])
                nc.scalar.dma_start(x0p_t[:, sl], x0p_d[:, sl])
                nc.sync.dma_start(x0c_t[:, sl], x0c_d[:, sl])

            # first 512 columns of x0 lead both queues (DMA semaphores
            # post ~2.5us after the transfer, so queue position directly
            # shifts the first matmul); weights follow
            chunk_dma(0, 512)
            wk_t = [wtile(wk_d, 0, 128, BF16, "wk0"),
                    wtile(wk_d, 128, 128, BF16, "wk1"),
                    wtile(wkp_d, 0, 128, BF16, "wk2", eng=nc.scalar)]
            chunk_dma(512, 512)
            wv_t = [wtile(wv_d, 0, 128, BF16, "wv0", eng=nc.scalar),
                    wtile(wv_d, 128, 128, BF16, "wv1", eng=nc.scalar),
                    wtile(wvc_d, 0, 3, BF16, "wv2", eng=nc.scalar)]

            k_sb = [[None] * 8 for _ in range(2)]
            vT_sb = [None] * 32

            def emit_pair(nbp):
                for nb in (2 * nbp, 2 * nbp + 1):
                    sl = slice(NBLK * nb, NBLK * (nb + 1))
                    for kt in range(2):
                        kts = slice(128 * kt, 128 * (kt + 1))
                        pk = ps_sc.tile([128, NBLK], F32, tag="sc",
                                        name=f"pk{kt}_{nb}")
                        nc.tensor.matmul(pk[:], wk_t[0][:, kts],
                                         x0_t[0][:, sl], start=True,
                                         stop=False)
                        nc.tensor.matmul(pk[:], wk_t[1][:, kts],
                                         x0_t[1][:, sl], start=False,
                                         stop=False)
                        nc.tensor.matmul(pk[:], wk_t[2][:, kts],
                                         x0p_t[:, sl], start=False,
                                         stop=True)
                        kt_sb = kqv_p.tile([128, NBLK], BF16,
                                           tag=f"k{kt}_{nb}",
                                           name=f"k{kt}_{nb}")
                        nc.scalar.activation(kt_sb[:], pk[:], Act.Relu)
                        k_sb[kt][nb] = kt_sb
                for nb in (2 * nbp, 2 * nbp + 1):
                    for sub in range(4):
                        i = 4 * nb + sub
                        ss = slice(128 * i, 128 * (i + 1))
                        pv = ps_sc.tile([128, K], F32, tag="sc",
                                        name=f"pv{i}")
                        nc.tensor.matmul(pv[:], x0_t[0][:, ss], wv_t[0][:],
                                         start=True, stop=False)
                        nc.tensor.matmul(pv[:], x0_t[1][:, ss], wv_t[1][:],
                                         start=False, stop=False)
                        nc.tensor.matmul(pv[:], x0c_t[:, ss], wv_t[2][:],
                                         start=False, stop=True)
                        vt_sb = kqv_p.tile([128, K], BF16, tag=f"v{i}",
                                           name=f"v{i}")
                        nc.vector.tensor_scalar_max(vt_sb[:], pv[:], 0.0)
                        vT_sb[i] = vt_sb

            q_sb = [[None] * NSLOT for _ in range(2)]

            def emit_q(s):
                sl = slice(NBLK * s, NBLK * (s + 1))
                for kt in range(2):
                    kts = slice(128 * kt, 128 * (kt + 1))
                    pq = ps_sc.tile([128, NBLK], F32, tag="sc",
                                    name=f"pq{kt}_{s}")
                    nc.tensor.matmul(pq[:], wq_t[0][:, kts], xq_t[0][:, sl],
                                     start=True, stop=False)
                    nc.tensor.matmul(pq[:], wq_t[1][:, kts], xq_t[1][:, sl],
                                     start=False, stop=False)
                    nc.tensor.matmul(pq[:], wq_t[2][:, kts], xqp_t[:, sl],
                                     start=False, stop=True)
                    qt = kqv_p.tile([128, NBLK], BF16, tag=f"q{kt}_{s}",
                                    name=f"q{kt}_{s}")
                    nc.scalar.activation(qt[:], pq[:], Act.Relu)
                    q_sb[kt][s] = qt

            slot_state = {}  # s -> (po, pd)

            def emit_slot(s, pending=None):
                """scores, exp, masks, den quads, attn@v for slot s;
                finalize(pending) is emitted after the first quad so its
                PSUM banks free early and the fc work hides in this
                slot's score stream."""
                M = M_S[s]
                po = [ps_o.tile([128, NBLK], F32, tag=f"o{vt}",
                                name=f"po{vt}_{s}") for vt in range(2)]
                pd = ps_d1.tile([128, NBLK], F32, tag="d1", name=f"pd{s}")
                mk_tiles = []
                for t in range(8):
                    mk = msk_p.tile([128, NBLK], BF16, tag="mk",
                                    name=f"mk{s}_{t}")
                    nc.scalar.dma_start(mk[:], msk_d[s, t])
                    mk_tiles.append(mk)
                ex_tiles = [None] * M
                for i in range(M):
                    psc = ps_sc.tile([128, NBLK], F32, tag="sc",
                                     name=f"psc{s}_{i}")
                    for kt in range(2):
                        nc.tensor.matmul(
                            psc[:],
                            k_sb[kt][i // 4][:, 128 * (i % 4):128 * (i % 4 + 1)],
                            q_sb[kt][s][:], start=(kt == 0), stop=(kt == 1))
                    ex = ex_p.tile([128, NBLK], BF16, tag="ex",
                                   name=f"ex{s}_{i}")
                    nc.scalar.activation(ex[:], psc[:], Act.Exp)
                    if i >= M - 8:
                        nc.vector.tensor_mul(ex[:], ex[:],
                                             mk_tiles[i - (M - 8)][:])
                    ex_tiles[i] = ex
                    if i % 4 == 3:
                        j = i - 3
                        if j == 0 and pending is not None:
                            finalize(pending)
                        # attn@v for the finished quad
                        for jj in range(j, j + 4):
                            e = ex_tiles[jj]
                            for vt in range(2):
                                nc.tensor.matmul(
                                    po[vt][:],
                                    vT_sb[jj][:, 128 * vt:128 * (vt + 1)],
                                    e[:], start=(jj == 0), stop=(jj == M - 1))
                        # quad-summed denominator
                        da = ds_p.tile([128, NBLK], BF16, tag="ds",
                                       name=f"da{s}_{j}")
                        nc.vector.tensor_add(da[:], ex_tiles[j][:],
                                             ex_tiles[j + 1][:])
                        db = ds_p.tile([128, NBLK], BF16, tag="ds",
                                       name=f"db{s}_{j}")
                        nc.vector.tensor_add(db[:], ex_tiles[j + 2][:],
                                             ex_tiles[j + 3][:])
                        dsum = ds_p.tile([128, NBLK], BF16, tag="ds",
                                         name=f"ds{s}_{j}")
                        nc.vector.tensor_add(dsum[:], da[:], db[:])
                        nc.tensor.matmul(pd[:], od_t[:], dsum[:],
                                         start=(j == 0), stop=(j == M - 4))
                        for jj in range(j, j + 4):
                            ex_tiles[jj] = None
                slot_state[s] = (po, pd)

            def finalize(s, halves=1):
                """normalize, fc, relu, residual, dma out for slot s.
                halves=2 pipelines the chain in 256-col pieces (used on
                the last slot, where this chain is the kernel tail)."""
                po, pd = slot_state[s]
                hw = NBLK // halves
                for h in range(halves):
                    hs = slice(h * hw, (h + 1) * hw)
                    rb = f_p.tile([128, hw], F32, tag="rb",
                                  name=f"rb{s}_{h}")
                    nc.vector.reciprocal_approx_fast(rb[:], pd[:, hs])
                    o_sb = []
                    for vt in range(2):
                        ot = o_p.tile([128, hw], BF16, tag=f"ob{vt}",
                                      name=f"ob{vt}_{s}_{h}")
                        nc.vector.tensor_mul(ot[:], po[vt][:, hs], rb[:])
                        o_sb.append(ot)
                    for ot in range(2):
                        pfc = ps_fc.tile([128, hw], F32, tag="fc",
                                         name=f"pfc{ot}_{s}_{h}")
                        for vt in range(2):
                            nc.tensor.matmul(
                                pfc[:],
                                fcw_t[vt][:, 128 * ot:128 * (ot + 1)],
                                o_sb[vt][:], start=(vt == 0),
                                stop=(vt == 1))
                        t_sb = tr_p.tile([128, hw], F32, tag=f"t{ot}",
                                         name=f"t{ot}_{s}_{h}")
                        nc.scalar.activation(t_sb[:], pfc[:], Act.Relu,
                                             bias=fcb_t[ot][:])
                        r_sb = tr_p.tile([128, hw], F32, tag=f"r{ot}",
                                         name=f"r{ot}_{s}_{h}")
                        nc.vector.tensor_add(
                            r_sb[:], t_sb[:],
                            xres_t[ot][:, NBLK * s + h * hw:
                                        NBLK * s + (h + 1) * hw])
                        nc.sync.dma_start(
                            out_d[128 * ot:128 * (ot + 1),
                                  NBLK * s + h * hw:
                                  NBLK * s + (h + 1) * hw], r_sb[:])

            # ---- emission schedule: spread k/v pairs between slots so
            # DMA and projections overlap attention; all bf16, no dtype
            # switches anywhere.
            emit_pair(0)

            # q inputs next on the queues (q runs right after pair 0) ...
            wq_t = [wtile(wq_d, 0, 128, BF16, "wq0"),
                    wtile(wq_d, 128, 128, BF16, "wq1"),
                    wtile(wqp_d, 0, 128, BF16, "wq2")]
            xq_t = [wtile(xq_d, 0, 128, BF16, "xq0"),
                    wtile(xq_d, 128, 128, BF16, "xq1")]
            xqp_t = wtile(xqp_d, 0, 128, BF16, "xqp")
            fcb_t = [wtile(fcb_d, 0, 128, F32, "fcb0"),
                     wtile(fcb_d, 128, 128, F32, "fcb1")]
            od_t = wtile(od_d, 0, 128, BF16, "onesd")

            emit_q(0)
            emit_slot(0)
            chunk_dma(1024, 1024)
            fcw_t = [wtile(fcw_d, 0, 128, BF16, "fcw0", eng=nc.scalar),
                     wtile(fcw_d, 128, 128, BF16, "fcw1", eng=nc.scalar)]
            xres_t = [wtile(xres_d, 128 * ot, 128, F32, f"xres{ot}",
                            eng=nc.scalar) for ot in range(2)]
            emit_q(1)
            emit_pair(1)
            emit_slot(1, pending=0)
            chunk_dma(2048, 1024)
            emit_q(2)
            emit_pair(2)
            emit_slot(2, pending=1)
            chunk_dma(3072, 1024)
            emit_q(3)
            emit_pair(3)
            emit_slot(3, pending=2)
            finalize(3)

    nc.compile()
    return nc


def _host_prep(x, q_w, q_b, k_w, k_b, v_w, v_b, fc_w, fc_b):
    """Build the per-core input maps."""
    import ml_dtypes
    f32 = np.float32
    bf16 = ml_dtypes.bfloat16
    n = np.arange(N)
    px = ((n // S) / S).astype(f32)
    py = ((n % S) / S).astype(f32)
    pos3 = np.stack([px, py, np.ones(N, f32)])   # [3, N] incl bias channel

    pos_pad = np.zeros((128, N), f32)
    pos_pad[:3] = pos3

    def pad_w(w, b):
        # rows 0..1 = pos weight rows, row 2 = bias, rest zero
        p = np.zeros((128, K), f32)
        p[:2] = w.astype(f32).T[C:]
        p[2] = b.astype(f32)
        return p

    # per-role masks [NSLOT, 8, 128, 512]
    mm = np.arange(128)[:, None]
    nn = np.arange(NBLK)[None, :]
    masks = {}
    for r in range(2):
        mr = np.zeros((NSLOT, 8, 128, NBLK), f32)
        for s in range(NSLOT):
            j = BLOCKS[r][s]
            for t in range(8):
                i = M_S[s] - 8 + t
                mr[s, t] = (128 * i + mm <= 512 * j + nn)
        masks[r] = mr.astype(bf16)

    shared = {
        "wq": np.ascontiguousarray(q_w.astype(f32).T[:C]).astype(bf16),
        "wk": np.ascontiguousarray(k_w.astype(f32).T[:C]).astype(bf16),
        "wv": np.ascontiguousarray(v_w.astype(f32).T[:C]).astype(bf16),
        "wqp": pad_w(q_w, q_b).astype(bf16),
        "wkp": pad_w(k_w, k_b).astype(bf16),
        "wvc": np.ascontiguousarray(
            np.concatenate([v_w.astype(f32).T[C:],
                            v_b.astype(f32)[None, :]], 0)).astype(bf16),
        "x0p": pos_pad.astype(bf16),
        "x0c": pos3.astype(bf16),
        "fcw": np.ascontiguousarray(fc_w.astype(f32).T).astype(bf16),
        "fcb": np.ascontiguousarray(fc_b.astype(f32).reshape(C, 1)),
        "onesd": np.ones((128, 128), f32).astype(bf16),
    }

    in_maps = []
    for c in range(8):
        b, r = c // 2, c % 2
        xb = x[b].reshape(C, N).astype(f32)
        qcols = np.concatenate(
            [np.arange(NBLK * j, NBLK * (j + 1)) for j in BLOCKS[r]])
        in_maps.append(dict(
            shared,
            x0b=xb.astype(bf16),
            xq=np.ascontiguousarray(xb[:, qcols]).astype(bf16),
            xqp=np.ascontiguousarray(pos_pad[:, qcols]).astype(bf16),
            xres=np.ascontiguousarray(xb[:, qcols]),
            masks=masks[r],
        ))
    return in_maps


def _gather(results):
    out = np.empty((B, C, N), np.float32)
    for c in range(8):
        b, r = c // 2, c % 2
        oc = results[c]["out"]
        for s, j in enumerate(BLOCKS[r]):
            out[b][:, NBLK * j:NBLK * (j + 1)] = oc[:, NBLK * s:NBLK * (s + 1)]
    return out.reshape(B, C, S, S)


def run(trace=False, **inputs):
    from concourse import bass_utils
    global _PROGRAM
    if _PROGRAM is None:
        _PROGRAM = _build_program()
    in_maps = _host_prep(**inputs)
    res = bass_utils.run_bass_kernel_spmd(
        _PROGRAM, in_maps, list(range(8)), trace=trace)
    return _gather(res.results), res


def kernel(**inputs):
    out, _ = run(trace=False, **inputs)
    return out
